# revision 24
# baseline (speedup 1.0000x reference)
"""Trainium2 Bass kernel for nn_AttnDecoderRNN (Bahdanau attention + GRU decoder).

Sharding: data-parallel over batch (8 batches/core) for the recurrence +
attention; tensor-parallel over vocab (4000/core) for the output projection,
with an AllGather of the GRU outputs and an AllGather of softmax stats.

Self-contained: hardcodes shapes B=64, S=128, H=1024, V=32000, T=10, SOS=0.
"""
import os
import numpy as np
import ml_dtypes

import concourse.mybir as mybir
import concourse.tile as tile
from concourse import bacc
from concourse.bass import ds, ts
from concourse.bass_utils import run_bass_kernel_spmd
from concourse.masks import make_identity
from contextlib import ExitStack

BF16 = mybir.dt.bfloat16
F32 = mybir.dt.float32
AF = mybir.ActivationFunctionType
ALU = mybir.AluOpType
AX = mybir.AxisListType

B, S, H, V, T = 64, 128, 1024, 32000, 10
W = 8            # cores
BC = B // W      # batches per core (8)
VC = V // W      # vocab shard (4000)
HC = H // 128    # h chunks (8)
G3 = 3 * H       # gate width (3072)
NBT = BC * T     # gi_emb rows per core (80)
MT = B * T // 128  # projection M tiles (5)
SOS = 0

_CACHE = {}


def build_nc():
    nc = bacc.Bacc("TRN2", target_bir_lowering=False, debug=False, num_devices=W)

    # ---- inputs (per core) ----
    eoT = nc.dram_tensor("eoT", [BC, H, S], BF16, kind="ExternalInput")
    uaT = nc.dram_tensor("uaT", [H, H], BF16, kind="ExternalInput")
    uawa_rep = nc.dram_tensor("uawa_rep", [128, H], F32, kind="ExternalInput")
    wi_embT = nc.dram_tensor("wi_embT", [H, G3], BF16, kind="ExternalInput")
    embT = nc.dram_tensor("embT", [H, NBT], BF16, kind="ExternalInput")
    bias_rep80 = nc.dram_tensor("bias_rep80", [NBT, G3], F32, kind="ExternalInput")
    wi_ctxT = nc.dram_tensor("wi_ctxT", [H, G3], BF16, kind="ExternalInput")
    whT = nc.dram_tensor("whT", [H, G3], BF16, kind="ExternalInput")
    waT = nc.dram_tensor("waT", [H, H], BF16, kind="ExternalInput")
    eo_b = nc.dram_tensor("eo_b", [BC, S, H], BF16, kind="ExternalInput")
    va_rep = nc.dram_tensor("va_rep", [128, H], F32, kind="ExternalInput")
    h0T = nc.dram_tensor("h0T", [H, BC], BF16, kind="ExternalInput")
    h0_sb = nc.dram_tensor("h0_sb", [128, 256], F32, kind="ExternalInput")
    owT = nc.dram_tensor("owT", [H, VC], BF16, kind="ExternalInput")
    outb_rep = nc.dram_tensor("outb_rep", [128, VC], F32, kind="ExternalInput")

    # ---- outputs (per core) ----
    log_probs_o = nc.dram_tensor("log_probs", [B * T, VC], F32,
                                 kind="ExternalOutput")
    h_final_o = nc.dram_tensor("h_final", [128, 256], F32, kind="ExternalOutput")
    attns_o = nc.dram_tensor("attns", [T, BC, S], F32, kind="ExternalOutput")

    with tile.TileContext(nc) as tc, ExitStack() as ctx:
        dram = ctx.enter_context(tc.tile_pool(name="dram", bufs=1, space="DRAM"))
        giemb_d = dram.tile([NBT, G3], F32, tag="giemb")
        outs_bounce = dram.tile([T, H, BC], BF16, tag="outsb")
        ag_outs = dram.tile([W, T, H, BC], BF16, tag="agouts",
                            addr_space="Shared")
        q_d = dram.tile([BC, H], F32, tag="qd")
        stats_bounce = dram.tile([128, MT], F32, tag="statsb")
        ag_stats = dram.tile([W, 128, MT], F32, tag="agstats",
                             addr_space="Shared")

        cst = ctx.enter_context(tc.tile_pool(name="cst", bufs=1))
        t_ident = cst.tile([128, 128], F32, tag="ident")
        make_identity(nc, t_ident[:])

        with tc.tile_pool(name="res", bufs=1) as res:
            t_wic = res.tile([128, HC * G3], BF16, tag="wic")
            t_wh = res.tile([128, HC * G3], BF16, tag="wh")
            t_wa = res.tile([128, HC * H], BF16, tag="wa")
            t_uk = res.tile([128, BC * H], BF16, tag="uk")
            t_eo = res.tile([128, BC * H], BF16, tag="eo")
            t_va = res.tile([128, H], F32, tag="va")

            for c in range(HC):
                nc.sync.dma_start(t_wic[:, ts(c, G3)],
                                  wi_ctxT[ds(c * 128, 128), :])
                nc.sync.dma_start(t_wh[:, ts(c, G3)], whT[ds(c * 128, 128), :])
                nc.sync.dma_start(t_wa[:, ts(c, H)], waT[ds(c * 128, 128), :])
            nc.sync.dma_start(t_va[:], va_rep[:])
            nc.sync.dma_start(
                t_eo[:].rearrange("p (b j) -> p b j", b=BC),
                eo_b[:, :, :].rearrange("b p j -> p b j"))

            # ----------------------------------------------------------
            # phase A1: UK[b] = eo[b] @ Ua.T + (Ua_b + Wa_b)    [S, H]
            # ----------------------------------------------------------
            with tc.tile_pool(name="a1", bufs=1) as a1, \
                 tc.tile_pool(name="a1ps", bufs=2, space="PSUM") as a1ps:
                t_eoT = a1.tile([128, BC * HC * S], BF16, tag="eoT")
                t_uaT = a1.tile([128, HC * H], BF16, tag="uaT")
                t_rep = a1.tile([128, H], F32, tag="rep")
                for b in range(BC):
                    nc.sync.dma_start(
                        t_eoT[:, ds(b * HC * S, HC * S)].rearrange(
                            "p (c s) -> p c s", c=HC),
                        eoT[b].rearrange("(c p) s -> p c s", p=128))
                for c in range(HC):
                    nc.sync.dma_start(t_uaT[:, ts(c, H)],
                                      uaT[ds(c * 128, 128), :])
                nc.sync.dma_start(t_rep[:], uawa_rep[:])

                for b in range(BC):
                    for half in range(2):
                        puk = a1ps.tile([128, 512], F32, tag="puk")
                        for c in range(HC):
                            nc.tensor.matmul(
                                puk[:],
                                t_eoT[:, ds((b * HC + c) * S, S)],
                                t_uaT[:, ds(c * H + half * 512, 512)],
                                start=(c == 0), stop=(c == HC - 1))
                        nc.vector.scalar_tensor_tensor(
                            t_uk[:, ds(b * H + half * 512, 512)],
                            puk[:], 1.0, t_rep[:, ds(half * 512, 512)],
                            op0=ALU.mult, op1=ALU.add)

            # ----------------------------------------------------------
            # phase A2: gi_emb = emb @ Wi_emb.T + (bi + bh) -> DRAM
            # ----------------------------------------------------------
            with tc.tile_pool(name="a2", bufs=1) as a2, \
                 tc.tile_pool(name="a2b", bufs=1) as a2b, \
                 tc.tile_pool(name="a2ps", bufs=1, space="PSUM") as a2ps:
                t_embT = a2.tile([128, HC * NBT], BF16, tag="embT")
                t_bias = a2.tile([NBT, G3], F32, tag="bias80")
                nc.sync.dma_start(t_bias[:], bias_rep80[:])
                for c in range(HC):
                    nc.sync.dma_start(t_embT[:, ts(c, NBT)],
                                      embT[ds(c * 128, 128), :])
                pges = [a2ps.tile([NBT, 512], F32, tag=f"pge{sl}",
                                  name=f"pge{sl}") for sl in range(6)]
                for half in range(2):
                    t_wie = a2b.tile([128, 4 * G3], BF16, tag="wie")
                    for cc in range(4):
                        c = half * 4 + cc
                        nc.sync.dma_start(t_wie[:, ts(cc, G3)],
                                          wi_embT[ds(c * 128, 128), :])
                    for sl in range(6):
                        for cc in range(4):
                            c = half * 4 + cc
                            nc.tensor.matmul(
                                pges[sl][:], t_embT[:, ts(c, NBT)],
                                t_wie[:, ds(cc * G3 + sl * 512, 512)],
                                start=(c == 0), stop=(c == HC - 1))
                for sl in range(6):
                    t_ge = a2.tile([NBT, 512], F32, tag="ge")
                    nc.vector.scalar_tensor_tensor(
                        t_ge[:], pges[sl][:], 1.0,
                        t_bias[:, ds(sl * 512, 512)],
                        op0=ALU.mult, op1=ALU.add)
                    nc.sync.dma_start(giemb_d[:, ds(sl * 512, 512)], t_ge[:])

            # ----------------------------------------------------------
            # recurrence (10 steps)
            # ----------------------------------------------------------
            with tc.tile_pool(name="rp", bufs=2) as rp, \
                 tc.tile_pool(name="rp3", bufs=2) as rp3, \
                 tc.tile_pool(name="rp1", bufs=1) as rp1, \
                 tc.tile_pool(name="ps_q", bufs=1, space="PSUM") as ps_q, \
                 tc.tile_pool(name="ps_g", bufs=1, space="PSUM") as ps_g, \
                 tc.tile_pool(name="ps_t", bufs=2, space="PSUM") as ps_t:
                t_hT = rp.tile([128, HC * BC], BF16, tag="hT")
                nc.sync.dma_start(
                    t_hT[:].rearrange("p (c b) -> p c b", c=HC),
                    h0T[:, :].rearrange("(c p) b -> p c b", p=128))
                t_hsb = rp.tile([128, 256], F32, tag="hsb0")
                nc.sync.dma_start(t_hsb[:], h0_sb[:])

                NSTEP = int(os.environ.get("K_STEPS", T))
                KCUT = int(os.environ.get("K_CUT", 99))
                for t in range(NSTEP):
                    # gi_emb slice for step t: [128 (4 strips x 8 b), 768]
                    t_ge_t = rp.tile([128, 768], F32, tag="ge_t")
                    nc.vector.memset(t_ge_t[:], 0.0)
                    gv = giemb_d[ds(t * BC, BC), :].rearrange(
                        "b (g s4 jj) -> b g s4 jj", g=3, s4=4)
                    for s4 in range(4):
                        nc.sync.dma_start(
                            t_ge_t[ds(32 * s4, BC), :].rearrange(
                                "b (g jj) -> b g jj", g=3),
                            gv[:, :, s4, :])

                    # q = h @ Wa.T -> [8, 1024]
                    p_q = ps_q.tile([BC, H], F32, tag="q")
                    for qh in range(2):
                        for c in range(HC):
                            nc.tensor.matmul(
                                p_q[:, ds(qh * 512, 512)],
                                t_hT[:, ts(c, BC)],
                                t_wa[:, ds(c * H + qh * 512, 512)],
                                start=(c == 0), stop=(c == HC - 1))
                    t_q = rp1.tile([BC, H], F32, tag="qsb")
                    nc.vector.tensor_copy(t_q[:], p_q[:])
                    nc.sync.dma_start(q_d[:], t_q[:])

                    if KCUT < 2:
                        continue
                    # attention: scores[b] = va . tanh(q[b] + UK[b])
                    t_sc = rp.tile([128, BC], F32, tag="scores")
                    for b in range(BC):
                        t_tanh = rp3.tile([128, H], F32, tag="tanh")
                        nc.gpsimd.dma_start(
                            t_tanh[:], q_d[ds(b, 1), :].broadcast_to([128, H]))
                        nc.vector.scalar_tensor_tensor(
                            t_tanh[:], t_uk[:, ts(b, H)], 1.0, t_tanh[:],
                            op0=ALU.mult, op1=ALU.add)
                        nc.scalar.activation(t_tanh[:], t_tanh[:], AF.Tanh)
                        nc.vector.scalar_tensor_tensor(
                            t_tanh[:], t_tanh[:], 1.0, t_va[:],
                            op0=ALU.mult, op1=ALU.mult,
                            accum_out=t_sc[:, ds(b, 1)])

                    if KCUT < 3:
                        continue
                    # softmax over S (via transpose to rows)
                    p_scT = ps_t.tile([BC, S], F32, tag="tp")
                    nc.tensor.transpose(p_scT[:], t_sc[:], t_ident[:])
                    t_negmax = rp.tile([BC, 1], F32, tag="negmax")
                    nc.vector.tensor_reduce(t_negmax[:], p_scT[:], axis=AX.X,
                                            op=ALU.max, negate=True)
                    t_w = rp.tile([BC, S], F32, tag="wrow")
                    t_sumexp = rp.tile([BC, 1], F32, tag="sumexp")
                    nc.scalar.activation(t_w[:], p_scT[:], AF.Exp,
                                         bias=t_negmax[:],
                                         accum_out=t_sumexp[:])
                    t_rinv = rp.tile([BC, 1], F32, tag="rinv")
                    nc.vector.reciprocal(t_rinv[:], t_sumexp[:])
                    t_attn = rp.tile([BC, S], F32, tag="attnrow")
                    nc.vector.tensor_scalar(t_attn[:], t_w[:], t_rinv[:],
                                            None, op0=ALU.mult)
                    nc.sync.dma_start(attns_o[t], t_attn[:])
                    p_wT = ps_t.tile([128, BC], F32, tag="tp")
                    nc.tensor.transpose(p_wT[:], t_attn[:],
                                        t_ident[ds(0, BC), ds(0, BC)])
                    t_wbf = rp.tile([128, BC], BF16, tag="wbf")
                    nc.vector.tensor_copy(t_wbf[:], p_wT[:])

                    if KCUT < 4:
                        continue
                    # ctx.T chunks [128 jj, 8 b] packed into [128, 64]
                    p_ctxT = ps_g.tile([128, HC * BC], F32, tag="ctxT")
                    for b in range(BC):
                        for u in range(HC):
                            nc.tensor.matmul(
                                p_ctxT[:, ds(u * BC + b, 1)],
                                t_eo[:, ds(b * H + u * 128, 128)],
                                t_wbf[:, ds(b, 1)],
                                start=True, stop=True)
                    t_ctxT = rp.tile([128, HC * BC], BF16, tag="ctxTbf")
                    nc.vector.tensor_copy(t_ctxT[:], p_ctxT[:])

                    if KCUT < 5:
                        continue
                    # gates
                    p_rz = ps_g.tile([128, 512], F32, tag="rz")
                    p_nn = ps_g.tile([128, 512], F32, tag="nn")
                    nc.vector.memset(p_rz[:], 0.0)
                    nc.vector.memset(p_nn[:], 0.0)
                    for s4 in range(4):
                        tp = (0, 32 * s4)
                        out_rz = p_rz[ds(32 * s4, BC), :]
                        for c in range(HC):
                            wic3 = t_wic[:, ds(c * G3, 2 * H)].rearrange(
                                "p (g j) -> p g j", g=2)
                            wh3 = t_wh[:, ds(c * G3, 2 * H)].rearrange(
                                "p (g j) -> p g j", g=2)
                            nc.tensor.matmul(
                                out_rz, t_ctxT[:, ts(c, BC)],
                                wic3[:, :, ds(s4 * 256, 256)],
                                start=(c == 0), stop=False,
                                skip_group_check=True, tile_position=tp)
                            nc.tensor.matmul(
                                out_rz, t_hT[:, ts(c, BC)],
                                wh3[:, :, ds(s4 * 256, 256)],
                                start=False, stop=(c == HC - 1),
                                skip_group_check=True, tile_position=tp)
                        for c in range(HC):
                            nc.tensor.matmul(
                                p_nn[ds(32 * s4, BC), ds(0, 256)],
                                t_ctxT[:, ts(c, BC)],
                                t_wic[:, ds(c * G3 + 2 * H + s4 * 256, 256)],
                                start=(c == 0), stop=(c == HC - 1),
                                skip_group_check=True, tile_position=tp)
                        for c in range(HC):
                            nc.tensor.matmul(
                                p_nn[ds(32 * s4, BC), ds(256, 256)],
                                t_hT[:, ts(c, BC)],
                                t_wh[:, ds(c * G3 + 2 * H + s4 * 256, 256)],
                                start=(c == 0), stop=(c == HC - 1),
                                skip_group_check=True, tile_position=tp)

                    if KCUT < 6:
                        continue
                    # gate math: sigmoid(x) = 0.5*tanh(x/2) + 0.5
                    t_rzs = rp1.tile([128, 512], F32, tag="rzs")
                    nc.vector.scalar_tensor_tensor(
                        t_rzs[:], p_rz[:], 1.0, t_ge_t[:, ds(0, 512)],
                        op0=ALU.mult, op1=ALU.add)
                    t_trz = rp1.tile([128, 512], F32, tag="trz")
                    nc.scalar.activation(t_trz[:], t_rzs[:], AF.Tanh,
                                         scale=0.5)
                    t_tmp1 = rp.tile([128, 256], F32, tag="tmp1")
                    nc.vector.scalar_tensor_tensor(
                        t_tmp1[:], t_trz[:, ds(0, 256)], 1.0,
                        p_nn[:, ds(256, 256)], op0=ALU.add, op1=ALU.mult)
                    t_gin = rp.tile([128, 256], F32, tag="gins")
                    nc.vector.scalar_tensor_tensor(
                        t_gin[:], p_nn[:, ds(0, 256)], 1.0,
                        t_ge_t[:, ds(512, 256)], op0=ALU.mult, op1=ALU.add)
                    t_narg = rp.tile([128, 256], F32, tag="narg")
                    nc.vector.scalar_tensor_tensor(
                        t_narg[:], t_tmp1[:], 0.5, t_gin[:],
                        op0=ALU.mult, op1=ALU.add)
                    t_n = rp.tile([128, 256], F32, tag="ngate")
                    nc.scalar.activation(t_n[:], t_narg[:], AF.Tanh)
                    # h_new = 0.5*(n + h) + 0.5 * t_z * (h - n)
                    t_a = rp.tile([128, 256], F32, tag="hmn")
                    nc.vector.scalar_tensor_tensor(
                        t_a[:], t_hsb[:], 1.0, t_n[:],
                        op0=ALU.mult, op1=ALU.subtract)
                    t_b2 = rp.tile([128, 256], F32, tag="zhm")
                    nc.vector.scalar_tensor_tensor(
                        t_b2[:], t_trz[:, ds(256, 256)], 1.0, t_a[:],
                        op0=ALU.mult, op1=ALU.mult)
                    t_c2 = rp.tile([128, 256], F32, tag="nph")
                    nc.vector.scalar_tensor_tensor(
                        t_c2[:], t_n[:], 1.0, t_hsb[:],
                        op0=ALU.mult, op1=ALU.add)
                    t_hnew = rp.tile([128, 256], F32, tag="hsb")
                    nc.vector.scalar_tensor_tensor(
                        t_hnew[:], t_b2[:], 1.0, t_c2[:],
                        op0=ALU.mult, op1=ALU.add)
                    nc.scalar.mul(t_hnew[:], t_hnew[:], 0.5)

                    if KCUT < 7:
                        continue
                    # h_new -> hT chunks (bf16) + outs bounce
                    t_hT = rp.tile([128, HC * BC], BF16, tag="hT")
                    for u in range(2):
                        p_hTu = ps_g.tile([128, 128], F32, tag="hTps",
                                          name=f"phT{u}")
                        nc.tensor.transpose(
                            p_hTu[:], t_hnew[:, ds(u * 128, 128)],
                            t_ident[:])
                        # gather live columns 32*s4+b -> chunk c = 2*s4+u
                        nc.vector.tensor_copy(
                            t_hT[:, ds(8 * u, 0)].rearrange("p x -> p x")
                            if False else
                            t_hT[:].rearrange(
                                "p (s4 u b) -> p s4 u b", s4=4, u=2)[:, :, u, :],
                            p_hTu[:].rearrange(
                                "p (s4 g) -> p s4 g", s4=4)[:, :, ds(0, BC)])
                    if int(os.environ.get("K_SUB", 9)) >= 2:
                        nc.sync.dma_start(
                            outs_bounce[t].rearrange("(c kk) b -> kk c b",
                                                     kk=128),
                            t_hT[:].rearrange("p (c b) -> p c b", c=HC))
                    t_hsb = t_hnew
                    if t == NSTEP - 1:
                        nc.sync.dma_start(h_final_o[:], t_hnew[:])

        # ------------------------------------------------------------------
        # AllGather outs; projection; log_softmax
        # ------------------------------------------------------------------
        skip_proj = os.environ.get("K_STAGE") == "rec"
        if not skip_proj:
            nc.gpsimd.collective_compute(
                "AllGather", ALU.bypass,
                ins=[outs_bounce[:].opt()],
                outs=[ag_outs[:].opt()],
                replica_groups=[list(range(W))],
            )

        with tc.tile_pool(name="pj", bufs=1) as pj, \
             tc.tile_pool(name="pj2", bufs=2) as pj2, \
             tc.tile_pool(name="pjps", bufs=2, space="PSUM") as pjps:
          if not skip_proj:
            t_ow = pj.tile([128, HC * VC], BF16, tag="ow")
            for c in range(HC):
                nc.sync.dma_start(t_ow[:, ts(c, VC)], owT[ds(c * 128, 128), :])
            t_ob = pj.tile([128, VC], F32, tag="outb")
            nc.sync.dma_start(t_ob[:], outb_rep[:])
            t_oT = pj.tile([128, HC * 640], BF16, tag="outsT")
            oT_v = t_oT[:].rearrange("p (c t r bb) -> p c t r bb",
                                     c=HC, t=T, r=W)
            for c in range(HC):
                for r_ in range(W):
                    nc.sync.dma_start(
                        oT_v[:, c, :, r_, :],
                        ag_outs[r_, :, ds(c * 128, 128), :].rearrange(
                            "t kk bb -> kk t bb"))

            t_logits = pj.tile([128, MT * VC], F32, tag="logits")
            t_sume = pj.tile([128, MT], F32, tag="sume")
            slab = [512] * 7 + [416]
            for mi in range(MT):
                t_acc = pj2.tile([128, 1], F32, tag="acc")
                for w_ in range(8):
                    off = 512 * w_
                    nvs = slab[w_]
                    p_l = pjps.tile([128, 512], F32, tag="pl")
                    for c in range(HC):
                        nc.tensor.matmul(
                            p_l[:, ds(0, nvs)],
                            t_oT[:, ds(c * 640 + mi * 128, 128)],
                            t_ow[:, ds(c * VC + off, nvs)],
                            start=(c == 0), stop=(c == HC - 1))
                    nc.vector.scalar_tensor_tensor(
                        t_logits[:, ds(mi * VC + off, nvs)],
                        p_l[:, ds(0, nvs)], 1.0, t_ob[:, ds(off, nvs)],
                        op0=ALU.mult, op1=ALU.add)
                    t_d = pj2.tile([128, 512], F32, tag="dump")
                    t_pa = pj2.tile([128, 1], F32, tag="pacc")
                    nc.scalar.activation(
                        t_d[:, ds(0, nvs)],
                        t_logits[:, ds(mi * VC + off, nvs)],
                        AF.Exp, accum_out=t_pa[:])
                    if w_ == 0:
                        nc.vector.tensor_copy(t_acc[:], t_pa[:])
                    else:
                        nc.vector.tensor_add(t_acc[:], t_acc[:], t_pa[:])
                nc.vector.tensor_copy(t_sume[:, ds(mi, 1)], t_acc[:])

            # stats exchange -> global lse
            nc.sync.dma_start(stats_bounce[:], t_sume[:])
            nc.gpsimd.collective_compute(
                "AllGather", ALU.bypass,
                ins=[stats_bounce[:].opt()],
                outs=[ag_stats[:].opt()],
                replica_groups=[list(range(W))],
            )
            t_ags = pj2.tile([128, W * MT], F32, tag="ags")
            nc.sync.dma_start(
                t_ags[:].rearrange("p (r m) -> p r m", r=W),
                ag_stats[:, :, :].rearrange("r p m -> p r m"))
            t_gsum = pj2.tile([128, MT], F32, tag="gsum")
            nc.vector.tensor_copy(t_gsum[:], t_ags[:, ds(0, MT)])
            for r_ in range(1, W):
                nc.vector.tensor_add(t_gsum[:], t_gsum[:],
                                     t_ags[:, ds(r_ * MT, MT)])
            t_lse = pj2.tile([128, MT], F32, tag="lse")
            nc.scalar.activation(t_lse[:], t_gsum[:], AF.Ln)

            for mi in range(MT):
                t_out = pj2.tile([128, VC], F32, tag="lpout")
                nc.vector.tensor_scalar(
                    t_out[:], t_logits[:, ds(mi * VC, VC)],
                    t_lse[:, ds(mi, 1)], None, op0=ALU.subtract)
                nc.sync.dma_start(log_probs_o[ds(mi * 128, 128), :], t_out[:])

    if not nc.is_finalized():
        nc.finalize()
    return nc


# ----------------------------------------------------------------------------
# host side
# ----------------------------------------------------------------------------
def _bf16(x):
    return np.ascontiguousarray(np.asarray(x).astype(ml_dtypes.bfloat16))


def _f32(x):
    return np.ascontiguousarray(np.asarray(x, dtype=np.float32))


def prep_inputs(encoder_outputs, encoder_hidden, target_tensor, embedding,
                Wa_w, Wa_b, Ua_w, Ua_b, Va_w, Va_b,
                gru_wi, gru_wh, gru_bi, gru_bh, out_w, out_b):
    encoder_outputs = _f32(encoder_outputs)
    encoder_hidden = _f32(encoder_hidden)
    target_tensor = np.asarray(target_tensor)
    embedding = _f32(embedding)
    Wa_w = _f32(Wa_w); Wa_b = _f32(Wa_b)
    Ua_w = _f32(Ua_w); Ua_b = _f32(Ua_b)
    Va_w = _f32(Va_w)
    gru_wi = _f32(gru_wi); gru_wh = _f32(gru_wh)
    gru_bi = _f32(gru_bi); gru_bh = _f32(gru_bh)
    out_w = _f32(out_w); out_b = _f32(out_b)

    tokens = np.concatenate(
        [np.full((B, 1), SOS, dtype=target_tensor.dtype),
         target_tensor[:, :T - 1]], axis=1)
    emb_seq = embedding[tokens]                       # [B, T, H]

    uawa = np.broadcast_to((Ua_b + Wa_b)[None, :], (128, H))
    va_rep = np.broadcast_to(Va_w[0][None, :], (128, H))
    bias = np.broadcast_to((gru_bi + gru_bh)[None, :], (NBT, G3))

    wi_embT = _bf16(gru_wi[:, :H].T)
    wi_ctxT = _bf16(gru_wi[:, H:].T)
    whT = _bf16(gru_wh.T)
    waT = _bf16(Wa_w.T)
    uaT = _bf16(Ua_w.T)
    owT_full = out_w.T
    h0 = encoder_hidden[0]

    in_maps = []
    for c in range(W):
        bs = slice(c * BC, (c + 1) * BC)
        eo_c = encoder_outputs[bs]
        emb_c = emb_seq[bs]
        embT_c = emb_c.transpose(2, 1, 0).reshape(H, NBT)  # cols m = t*8+b
        h0_c = h0[bs]
        h0_sb = np.zeros((128, 256), np.float32)
        r = h0_c.reshape(BC, 4, 256)
        for s4 in range(4):
            h0_sb[32 * s4:32 * s4 + BC] = r[:, s4, :]
        vs = slice(c * VC, (c + 1) * VC)
        in_maps.append({
            "eoT": _bf16(eo_c.transpose(0, 2, 1)),
            "uaT": uaT,
            "uawa_rep": _f32(uawa),
            "wi_embT": wi_embT,
            "embT": _bf16(embT_c),
            "bias_rep80": _f32(bias),
            "wi_ctxT": wi_ctxT,
            "whT": whT,
            "waT": waT,
            "eo_b": _bf16(eo_c),
            "va_rep": _f32(va_rep),
            "h0T": _bf16(h0_c.T),
            "h0_sb": h0_sb,
            "owT": _bf16(owT_full[:, vs]),
            "outb_rep": _f32(np.broadcast_to(out_b[vs][None, :], (128, VC))),
        })
    return in_maps


def kernel(**inputs):
    if "nc" not in _CACHE:
        _CACHE["nc"] = build_nc()
    nc = _CACHE["nc"]
    in_maps = prep_inputs(**inputs)
    res = run_bass_kernel_spmd(nc, in_maps, list(range(W)))
    rs = res.results

    log_probs = np.empty((B, T, V), np.float32)
    h_final = np.empty((B, H), np.float32)
    attns = np.empty((B, T, S), np.float32)
    for c in range(W):
        r = rs[c]
        log_probs[:, :, c * VC:(c + 1) * VC] = \
            r["log_probs"].reshape(T, B, VC).transpose(1, 0, 2)
        hsb = r["h_final"]
        for s4 in range(4):
            h_final[c * BC:(c + 1) * BC, s4 * 256:(s4 + 1) * 256] = \
                hsb[32 * s4:32 * s4 + BC]
        attns[c * BC:(c + 1) * BC] = r["attns"].transpose(1, 0, 2)
    return log_probs, h_final[None], attns
